# revision 2
# baseline (speedup 1.0000x reference)
# ANI-style species-routed MLP ensemble on 8 Trainium2 NeuronCores.
#
# Strategy: data-parallel over atoms with host-side MoE routing. Atoms are
# grouped by species on the host; each core receives a feature-major
# pre-transposed slab xt[384, NTOT] whose columns are (species-contiguous,
# padded) atoms, plus replicated per-species MLP weights. The device runs
# each species' dense MLP on its contiguous segment — no on-device routing,
# no cross-core communication. Per-atom energies are gathered/unpermuted on
# the host.
#
# CELU(y) = max(y, min(a*exp(y/a), a) - a). With x' = celu + a propagated
# instead of celu (folding -a into the next layer's bias), each layer's
# epilogue is: t = Exp(psum/a + bias_a) on ScalarE, then
# x' = max(psum + (b + a), min(t, a)) on VectorE.
import math

import numpy as np

import concourse.mybir as mybir
from concourse import bacc
from concourse.bass_utils import run_bass_kernel_spmd
from concourse.tile import TileContext

ALPHA = 0.1
N_CORES = 8
AEV_DIM = 384
SPECIES_DIMS = [
    [384, 160, 128, 96, 1],  # H = species 0
    [384, 144, 112, 96, 1],  # C = 1
    [384, 128, 112, 96, 1],  # N = 2
    [384, 128, 112, 96, 1],  # O = 3
]
SUB = 512  # matmul free-dim tile (fp32 max moving operand / one PSUM bank)
G = 2      # SUB-subchunks per pipeline group

_cache = {}
LAST_RESULTS = None


def _chunks(n, c=128):
    return [(i, min(c, n - i)) for i in range(0, n, c)]


def _build(Ks):
    """Build + schedule the SPMD Bass graph for per-species segment sizes Ks."""
    f32 = mybir.dt.float32
    Alu = mybir.AluOpType
    Exp = mybir.ActivationFunctionType.Exp
    Copy = mybir.ActivationFunctionType.Copy
    NTOT = sum(Ks)

    nc = bacc.Bacc("TRN2", target_bir_lowering=False, debug=False,
                   num_devices=N_CORES)
    xt = nc.declare_dram_parameter("xt", [AEV_DIM, NTOT], f32, isOutput=False)
    out = nc.declare_dram_parameter("out", [1, NTOT], f32, isOutput=True)
    wp, bmp, bap = {}, {}, {}
    for s in range(4):
        dims = SPECIES_DIMS[s]
        for l in range(4):
            din, dout = dims[l], dims[l + 1]
            wp[s, l] = nc.declare_dram_parameter(
                f"w{s}{l}", [din, dout], f32, isOutput=False)
            if l < 3:
                bmp[s, l] = nc.declare_dram_parameter(
                    f"bm{s}{l}", [dout, 1], f32, isOutput=False)
                bap[s, l] = nc.declare_dram_parameter(
                    f"ba{s}{l}", [dout, 1], f32, isOutput=False)

    xtv = xt.ap().rearrange("(a p) n -> a p n", p=128)  # [3, 128, NTOT]

    with TileContext(nc) as tc:
        with tc.tile_pool(name="const", bufs=1) as cpool, \
             tc.tile_pool(name="data", bufs=2) as dpool, \
             tc.tile_pool(name="psum", bufs=1, space="PSUM") as ppool:

            # --- resident weights + biases -------------------------------
            wsb, bmsb, basb = {}, {}, {}
            for s in range(4):
                dims = SPECIES_DIMS[s]
                for l in range(4):
                    din, dout = dims[l], dims[l + 1]
                    for ci, (k0, ksz) in enumerate(_chunks(din)):
                        t = cpool.tile([ksz, dout], f32, tag=f"w{s}_{l}_{ci}")
                        nc.sync.dma_start(out=t, in_=wp[s, l][k0:k0 + ksz, :])
                        wsb[s, l, ci] = t
                    if l < 3:
                        for mi, (m0, msz) in enumerate(_chunks(dout)):
                            tb = cpool.tile([msz, 1], f32, tag=f"bm{s}_{l}_{mi}")
                            nc.sync.dma_start(out=tb, in_=bmp[s, l][m0:m0 + msz, :])
                            bmsb[s, l, mi] = tb
                            ta = cpool.tile([msz, 1], f32, tag=f"ba{s}_{l}_{mi}")
                            nc.sync.dma_start(out=ta, in_=bap[s, l][m0:m0 + msz, :])
                            basb[s, l, mi] = ta

            def celu(ps, msz, cols, bm, ba, xout):
                """xout[:msz,:cols] = celu(ps + b) + ALPHA, from PSUM ps."""
                te = dpool.tile([msz, G * SUB], f32, tag="te", bufs=4)
                nc.scalar.activation(
                    out=te[:msz, :cols], in_=ps[:msz, :cols], func=Exp,
                    bias=ba[:, :], scale=1.0 / ALPHA)
                yb = dpool.tile([msz, G * SUB], f32, tag="yb", bufs=4)
                nc.vector.tensor_scalar(
                    out=yb[:msz, :cols], in0=ps[:msz, :cols],
                    scalar1=bm[:, :], scalar2=None, op0=Alu.add)
                nc.vector.scalar_tensor_tensor(
                    out=xout[:msz, :cols], in0=te[:msz, :cols], scalar=ALPHA,
                    in1=yb[:msz, :cols], op0=Alu.min, op1=Alu.max)

            # --- per-species segments ------------------------------------
            off = 0
            for s in range(4):
                dims = SPECIES_DIMS[s]
                h1, h2, h3 = dims[1], dims[2], dims[3]
                mlist0 = _chunks(h1)
                Kseg = Ks[s]
                ngroups = (Kseg + G * SUB - 1) // (G * SUB)
                for j in range(ngroups):
                    col0 = off + j * G * SUB
                    cols = min(G * SUB, off + Kseg - col0)
                    gj = cols // SUB

                    # L0 inputs: 3 feature-chunk tiles [128, cols]
                    x0 = []
                    for ci in range(3):
                        t = dpool.tile([128, G * SUB], f32, tag="x0", bufs=6)
                        nc.sync.dma_start(
                            out=t[:, :cols], in_=xtv[ci, :, col0:col0 + cols])
                        x0.append(t)

                    # L0 matmuls: accumulate over 3 feature chunks
                    ps0 = {}
                    for mi, (m0, msz) in enumerate(mlist0):
                        ps0[mi] = ppool.tile(
                            [128 if mi == 0 else 32, G * SUB], f32,
                            tag="pl0a" if mi == 0 else "pl0b",
                            name=f"ps0_{mi}")
                    for ci in range(3):
                        for mi, (m0, msz) in enumerate(mlist0):
                            lhs = wsb[s, 0, ci][:, m0:m0 + msz]
                            for g in range(gj):
                                nc.tensor.matmul(
                                    ps0[mi][:msz, g * SUB:(g + 1) * SUB],
                                    lhs, x0[ci][:, g * SUB:(g + 1) * SUB],
                                    start=(ci == 0), stop=(ci == 2))

                    # L0 epilogue -> x1 tiles (per m-chunk)
                    x1 = {}
                    for mi, (m0, msz) in enumerate(mlist0):
                        x1[mi] = dpool.tile(
                            [128 if mi == 0 else 32, G * SUB], f32,
                            tag="x1a" if mi == 0 else "x1b", bufs=2,
                            name=f"x1_{mi}")
                        celu(ps0[mi], msz, cols, bmsb[s, 0, mi], basb[s, 0, mi],
                             x1[mi])

                    # L1: contract over h1 chunks
                    ps1 = ppool.tile([128, G * SUB], f32, tag="pl1")
                    nk1 = len(mlist0)
                    for ci, (k0, ksz) in enumerate(mlist0):
                        for g in range(gj):
                            nc.tensor.matmul(
                                ps1[:h2, g * SUB:(g + 1) * SUB],
                                wsb[s, 1, ci], x1[ci][:ksz, g * SUB:(g + 1) * SUB],
                                start=(ci == 0), stop=(ci == nk1 - 1))
                    x2 = dpool.tile([128, G * SUB], f32, tag="x2", bufs=2)
                    celu(ps1, h2, cols, bmsb[s, 1, 0], basb[s, 1, 0], x2)

                    # L2: single chunk contraction (h2 <= 128)
                    ps2 = ppool.tile([96, G * SUB], f32, tag="pl23")
                    for g in range(gj):
                        nc.tensor.matmul(
                            ps2[:, g * SUB:(g + 1) * SUB],
                            wsb[s, 2, 0], x2[:h2, g * SUB:(g + 1) * SUB],
                            start=True, stop=True)
                    x3 = dpool.tile([96, G * SUB], f32, tag="x3", bufs=2)
                    celu(ps2, h3, cols, bmsb[s, 2, 0], basb[s, 2, 0], x3)

                    # L3: energies [1, cols]; final bias applied on host
                    ps3 = ppool.tile([1, G * SUB], f32, tag="pl23")
                    for g in range(gj):
                        nc.tensor.matmul(
                            ps3[0:1, g * SUB:(g + 1) * SUB],
                            wsb[s, 3, 0], x3[:, g * SUB:(g + 1) * SUB],
                            start=True, stop=True)
                    oc = dpool.tile([1, G * SUB], f32, tag="oc", bufs=3)
                    nc.scalar.activation(
                        out=oc[:1, :cols], in_=ps3[:1, :cols], func=Copy)
                    nc.sync.dma_start(
                        out=out[0:1, col0:col0 + cols], in_=oc[:1, :cols])
                off += Kseg

    nc.compile()
    return nc


def _plan(species):
    """Per-core routing: indices per (core, species), padded static sizes."""
    idx_s = [np.nonzero(species == s)[0] for s in range(4)]
    q = [(len(ix) + N_CORES - 1) // N_CORES for ix in idx_s]
    Ks = [max(SUB, ((qq + SUB - 1) // SUB) * SUB) for qq in q]
    return idx_s, q, Ks


def kernel(**inputs):
    global LAST_RESULTS
    aev = np.ascontiguousarray(np.asarray(inputs["aev"], dtype=np.float32))
    species = np.asarray(inputs["species"]).astype(np.int64)
    params = [inputs["params_H"], inputs["params_C"], inputs["params_N"],
              inputs["params_O"]]
    N = aev.shape[0]

    idx_s, q, Ks = _plan(species)
    NTOT = sum(Ks)
    offs = np.concatenate([[0], np.cumsum(Ks)]).astype(np.int64)

    key = tuple(Ks)
    if key not in _cache:
        _cache[key] = _build(Ks)
    nc = _cache[key]

    # per-species device params (replicated to all cores)
    shared = {}
    host_add = []
    for s in range(4):
        for l in range(4):
            W = np.asarray(params[s][l][0], dtype=np.float32)
            b = np.asarray(params[s][l][1], dtype=np.float32)
            eb = b if l == 0 else b - ALPHA * W.sum(axis=1)
            shared[f"w{s}{l}"] = np.ascontiguousarray(W.T)
            if l < 3:
                shared[f"bm{s}{l}"] = np.ascontiguousarray(
                    (eb + ALPHA)[:, None])
                shared[f"ba{s}{l}"] = np.ascontiguousarray(
                    (eb / ALPHA + math.log(ALPHA))[:, None])
            else:
                host_add.append(float(eb[0]))

    in_maps = []
    perms = []
    for i in range(N_CORES):
        segs = []
        for s in range(4):
            seg = idx_s[s][i * q[s]:(i + 1) * q[s]]
            padv = idx_s[s][0] if len(idx_s[s]) else 0
            pad = np.full(Ks[s] - len(seg), padv, dtype=np.int64)
            segs.append(np.concatenate([seg, pad]))
        perm = np.concatenate(segs)
        perms.append(perm)
        xt_i = np.ascontiguousarray(aev[perm].T)
        in_maps.append({"xt": xt_i, **shared})

    res = run_bass_kernel_spmd(nc, in_maps, core_ids=list(range(N_CORES)))
    LAST_RESULTS = res

    out_full = np.empty((N, 1), dtype=np.float32)
    for i in range(N_CORES):
        ocore = res.results[i]["out"][0]
        for s in range(4):
            seg = idx_s[s][i * q[s]:(i + 1) * q[s]]
            r = len(seg)
            if r:
                out_full[seg, 0] = ocore[offs[s]:offs[s] + r] + host_add[s]
    return out_full
